# revision 12
# baseline (speedup 1.0000x reference)
"""Diagonal-Gaussian KL loss on 8 Trainium2 NeuronCores.

KL(p || q) summed over batch, with diag covariances exp(sigma):
  0.5 * [ sum(sigma_q - sigma_p) + sum(exp(sigma_p - sigma_q))
          + sum((mu_q-mu_p)^2 * exp(-sigma_q)) - B*D ]

Data-parallel over the batch dim: each core reduces a [1024, 2048] shard of
the four inputs to three per-partition partial sums; the tiny final combine
(8 cores x 128 partitions x 3 terms) happens on the host in float64.

The four inputs are stacked host-side into one [4, ROWS, D] tensor so each
[128, 2048] row-tile arrives in a single 4MB DMA.

Raw-bass pipeline (explicit semaphores; Tile was not usable here because
this walrus build allows only ONE sem-wait per compute/DMA instruction and
Tile's scheduler routinely emits two):
  per row-tile i (8 per core), with a 3-slot DMA ring and 2-slot compute
  buffers:
    SYNC: big[i%3] <- DMA row-tile i            (waits: slot free)
    DVE : a = sigma_p - sigma_q
          d = mu_q - mu_p                        (+inc: big slot released)
          u = d * e3                             (waits: e3 ready)
    ACT : e3 = exp(-0.5*sigma_q)                 (+inc)
          id(a)   accum-> acc_a   (in-place, result discarded)
          exp(a)  accum-> acc_e   (in-place, result discarded)
          u^2     accum-> acc_m   (in-place)     (+inc)
  tail: DVE reduces acc_* [128,8] -> res [128,3], SYNC DMAs res out.
The kernel is HBM-bound (~32MB/core, ~90us at ~360GB/s); DVE (~55us) and
ACT (~65us) hide under the DMA stream.
"""

from contextlib import ExitStack

import numpy as np

import concourse.bass as bass
from concourse import mybir
from concourse.bass_utils import run_bass_kernel_spmd

B, D = 8192, 2048
NCORES = 8
ROWS = B // NCORES  # rows per core
P = 128  # SBUF partitions
NT = ROWS // P  # row-tiles per core

F32 = mybir.dt.float32


def _build_nc():
    nc = bass.Bass(trn_type="TRN2", target_bir_lowering=False)

    x = nc.dram_tensor("x", [4, ROWS, D], F32, kind="ExternalInput")
    out = nc.dram_tensor("out", [P, 3], F32, kind="ExternalOutput")

    Exp = mybir.ActivationFunctionType.Exp
    Square = mybir.ActivationFunctionType.Square
    Identity = mybir.ActivationFunctionType.Identity
    Alu = mybir.AluOpType
    X = mybir.AxisListType.X

    ctx = ExitStack()
    with ctx:
        big = [ctx.enter_context(nc.sbuf_tensor(f"big{k}", [P, 4 * D], F32)) for k in range(3)]
        a_b = [ctx.enter_context(nc.sbuf_tensor(f"a{j}", [P, D], F32)) for j in range(2)]
        d_b = [ctx.enter_context(nc.sbuf_tensor(f"d{j}", [P, D], F32)) for j in range(2)]
        u_b = [ctx.enter_context(nc.sbuf_tensor(f"u{j}", [P, D], F32)) for j in range(2)]
        e3_b = [ctx.enter_context(nc.sbuf_tensor(f"e3{j}", [P, D], F32)) for j in range(2)]
        acc_a = ctx.enter_context(nc.sbuf_tensor("acc_a", [P, NT], F32))
        acc_e = ctx.enter_context(nc.sbuf_tensor("acc_e", [P, NT], F32))
        acc_m = ctx.enter_context(nc.sbuf_tensor("acc_m", [P, NT], F32))
        res = ctx.enter_context(nc.sbuf_tensor("res", [P, 3], F32))

        ds = [ctx.enter_context(nc.semaphore(f"ds{k}")) for k in range(3)]
        v_sem = ctx.enter_context(nc.semaphore("v_sem"))
        a_sem = ctx.enter_context(nc.semaphore("a_sem"))
        out_sem = ctx.enter_context(nc.semaphore("out_sem"))

        # DRAM AP for row-tile i: partitions = rows r..r+127, free = (t, d).
        def x_tile_ap(i):
            return bass.AP(x, i * P * D, [[D, P], [ROWS * D, 4], [1, D]])

        with nc.Block() as block:

            @block.sync
            def _(sync):
                for i in range(NT):
                    k = i % 3
                    if i >= 3:
                        # big[k]'s previous tile released by both engines
                        sync.wait_ge(v_sem, 2 * (i - 3) + 1)
                        sync.wait_ge(a_sem, 2 * (i - 3) + 1)
                    sync.dma_start(big[k][:, :], x_tile_ap(i)).then_inc(ds[k], 16)
                sync.wait_ge(v_sem, 2 * NT + 1)  # res written
                sync.dma_start(out[:, :], res[:, :]).then_inc(out_sem, 16)
                sync.wait_ge(out_sem, 16)

            @block.vector
            def _(vector):
                for i in range(NT):
                    k, j = i % 3, i % 2
                    vector.wait_ge(ds[k], 16 * (i // 3 + 1))  # tile i arrived
                    if i >= 2:
                        # a[j]/u[j] freed by ACT of iter i-2
                        vector.wait_ge(a_sem, 2 * (i - 2) + 2)
                    sq_t = big[k][:, 0:D]
                    sp_t = big[k][:, D : 2 * D]
                    mq_t = big[k][:, 2 * D : 3 * D]
                    mp_t = big[k][:, 3 * D : 4 * D]
                    vector.tensor_sub(a_b[j][:, :], sp_t, sq_t)
                    vector.tensor_sub(d_b[j][:, :], mq_t, mp_t).then_inc(v_sem, 1)
                    vector.wait_ge(a_sem, 2 * i + 1)  # e3(i) ready
                    vector.tensor_mul(u_b[j][:, :], d_b[j][:, :], e3_b[j][:, :]).then_inc(v_sem, 1)
                vector.wait_ge(a_sem, 2 * NT)  # all accums final
                vector.tensor_reduce(res[:, 0:1], acc_a[:, :], axis=X, op=Alu.add)
                vector.tensor_reduce(res[:, 1:2], acc_e[:, :], axis=X, op=Alu.add)
                vector.tensor_reduce(res[:, 2:3], acc_m[:, :], axis=X, op=Alu.add).then_inc(v_sem, 1)

            @block.scalar
            def _(scalar):
                for i in range(NT):
                    k, j = i % 3, i % 2
                    scalar.wait_ge(ds[k], 16 * (i // 3 + 1))  # sigma_q(i) arrived
                    if i >= 2:
                        scalar.wait_ge(v_sem, 2 * (i - 2) + 2)  # e3[j] freed
                    scalar.activation(
                        e3_b[j][:, :], big[k][:, 0:D], Exp, scale=-0.5
                    ).then_inc(a_sem, 1)
                    scalar.wait_ge(v_sem, 2 * i + 1)  # a(i) ready
                    scalar.activation(
                        a_b[j][:, :], a_b[j][:, :], Identity,
                        accum_out=acc_a[:, i : i + 1],
                    )
                    scalar.activation(
                        a_b[j][:, :], a_b[j][:, :], Exp,
                        accum_out=acc_e[:, i : i + 1],
                    )
                    scalar.wait_ge(v_sem, 2 * i + 2)  # u(i) ready
                    scalar.activation(
                        u_b[j][:, :], u_b[j][:, :], Square,
                        accum_out=acc_m[:, i : i + 1],
                    ).then_inc(a_sem, 1)

    return nc


_NC = None


def _get_nc():
    global _NC
    if _NC is None:
        _NC = _build_nc()
    return _NC


def _run(inputs, **kw):
    full = np.stack(
        [
            np.asarray(inputs["sigma_q"], dtype=np.float32),
            np.asarray(inputs["sigma_p"], dtype=np.float32),
            np.asarray(inputs["mu_q"], dtype=np.float32),
            np.asarray(inputs["mu_p"], dtype=np.float32),
        ],
        axis=0,
    )  # [4, B, D]
    in_maps = [
        {"x": np.ascontiguousarray(full[:, c * ROWS : (c + 1) * ROWS, :])}
        for c in range(NCORES)
    ]
    return run_bass_kernel_spmd(_get_nc(), in_maps, core_ids=list(range(NCORES)), **kw)


def _combine(results):
    # [8, 128, 3] partial sums -> scalar, in f64 for a clean final reduction
    S = np.stack([r["out"] for r in results]).astype(np.float64)
    s_a = S[..., 0].sum()
    s_e = S[..., 1].sum()
    s_m = S[..., 2].sum()
    kl = 0.5 * (-s_a + s_e + s_m - B * D)
    return np.asarray(kl, dtype=np.float32)


def kernel(**inputs):
    return _combine(_run(inputs).results)


def run_traced(inputs, **kw):
    """test.py helper: returns (value, BassKernelResults) with profiling."""
    br = _run(inputs, trace=True, **kw)
    return _combine(br.results), br
